# revision 1
# baseline (speedup 1.0000x reference)
"""Canny edge detection Bass kernel for TRN2 — per-core half-image slab.

Layout: N-layout [rows=partitions, cols=free]. Per core: image b = core//2,
half h = core%2. Input slab x_slab [3, 616, 1024] (global rows
clip(512h-19 .. 512h+597)). 5 tiles; tile t: gray/x rows slab [122t, 122t+128),
gx/m2 rows slab [122t+3, 122t+125). Edge-rows e = slabrow-3 (e in [0,544),
global g = 512h-16+e, owned = e [16,528)).
"""
import contextlib

import numpy as np
import ml_dtypes

import concourse.bass as bass
import concourse.mybir as mybir
import concourse.tile as tile_mod
from concourse.tile import TileContext
from concourse.mybir import AluOpType as Op
from concourse.vector_clock import ScopedClock as _SC

F32 = mybir.dt.float32
BF16 = mybir.dt.bfloat16
U8 = mybir.dt.uint8
U16 = mybir.dt.uint16
U32 = mybir.dt.uint32

# ---------------- drain-split patch (walrus 1-wait limit) ----------------


def _patched_drain_and_barrier(self, tick_clock, wait_clock):
    drain_inst = self.nc.sync.drain()
    wait_clock.add_sem_waits(drain_inst.ins, _SC({None: tick_clock.global_clock}))
    si = drain_inst.ins.sync_info
    if si is not None and si.on_wait and len(si.on_wait) > 1:
        waits = list(si.on_wait)
        si.on_wait = waits[:1]
        for w in waits[1:]:
            nop = self.nc.sync.nop(nofuse=True)
            nop.ins.sync_info = mybir.SyncInfo(on_wait=[w], on_update=[])
    self.nc.all_engine_barrier()
    assert self.sems is not None
    popped = self.nc._tile_sem_poison_stack.pop()
    assert popped is self._sem_poison
    self.nc.clear_and_free_semaphores(list(self.sems.allocated().values()))
    self.nc.all_engine_barrier()


tile_mod.TileContext._drain_and_barrier = _patched_drain_and_barrier

# ---------------- custom DVE ops ----------------
from concourse.dve_spec import (  # noqa: E402
    Spec, Src0, Src1, C0, C1, Zero, maxx, select, lower,
)
import concourse.dve_spec as dve_spec  # noqa: E402
from concourse.dve_uop import DveOpSpec  # noqa: E402
import concourse.dve_ops as dve_ops  # noqa: E402

_REGISTERED = {}


def register_op(name, spec):
    if name in _REGISTERED:
        return _REGISTERED[name]
    op = dve_ops.DveOp(name, spec, False, uops_sha={})
    idx = dve_ops._CUSTOM_DVE_ROW_BASE + len(dve_ops.OPS)
    assert idx < 0x20, "opcode rows exhausted"
    dve_ops.OPS.append(op)
    dve_ops.CUSTOM_DVE_SPECS[name] = spec
    dve_ops._SUB_OPCODE_FOR_NAME[name] = idx
    for ver in ("v3", "v4"):
        s = DveOpSpec(name=name, opcode=idx, uops=lower(spec, ver=ver),
                      rd1_en=dve_spec._has_src1(spec))
        op.uops_sha[ver] = s.sha(ver)
    _REGISTERED[name] = op
    return op


OP_MULADD2 = register_op("CN_MULADD2", Spec(
    body=Src0 * C0 + Src1 * C1,
    reference=lambda in0, in1, s0, s1, imm2:
        ((in0 * np.float32(s0)).astype(np.float32)
         + (in1 * np.float32(s1)).astype(np.float32)).astype(np.float32)))

OP_MAG2 = register_op("CN_MAG2", Spec(
    body=Src0 * Src0 + Src1 * Src1,
    reference=lambda in0, in1, s0, s1, imm2:
        ((in0 * in0).astype(np.float32)
         + (in1 * in1).astype(np.float32)).astype(np.float32)))

OP_AXCMP = register_op("CN_AXCMP", Spec(
    body=(maxx(Src0, Zero - Src0) * C0 >= maxx(Src1, Zero - Src1)),
    reference=lambda in0, in1, s0, s1, imm2:
        ((np.abs(in0) * np.float32(s0)).astype(np.float32)
         >= np.abs(in1)).astype(np.float32)))

OP_SGN = register_op("CN_SGN", Spec(
    body=(Src0 * Src1 > Zero),
    reference=lambda in0, in1, s0, s1, imm2:
        ((in0 * in1).astype(np.float32) > 0).astype(np.float32)))

OP_MAXM = register_op("CN_MAXM", Spec(
    body=maxx(Src0 * C0, Src1 * C1),
    reference=lambda in0, in1, s0, s1, imm2:
        np.maximum((in0 * np.asarray(s0, np.float32)).astype(np.float32),
                   (in1 * np.asarray(s1, np.float32)).astype(np.float32))
        .astype(np.float32)))

OP_ZSEL = register_op("CN_ZSEL", Spec(
    body=select(Src0 > Src1, Src0 + C0, Zero),
    reference=lambda in0, in1, s0, s1, imm2:
        np.where(in0 > in1, (in0 + np.float32(s0)).astype(np.float32),
                 np.float32(0.0)).astype(np.float32)))

# ---------------- multi-wait splitter (walrus allows 1 wait/inst) -------
import bass_rust as _br

_WSPLIT_CNT = [0]


def _split_multi_waits(nc):
    for f in nc.m.functions:
        for blk in f.blocks:
            new = []
            changed = False
            for ins in blk.instructions:
                si = ins.sync_info
                if si is not None and si.on_wait and len(si.on_wait) > 1:
                    waits = list(si.on_wait)
                    for w in waits[:-1]:
                        nop = _br.InstNoOp(
                            name=f"I-wsplit{_WSPLIT_CNT[0]}", ins=[], outs=[])
                        _WSPLIT_CNT[0] += 1
                        nop.engine = ins.engine
                        nop.bass_nofuse = True
                        nop.sync_info = mybir.SyncInfo(on_wait=[w],
                                                       on_update=[])
                        nc.register_instruction(nop, overwrite=True)
                        new.append(nop)
                    si.on_wait = waits[-1:]
                    changed = True
                new.append(ins)
            if changed:
                blk.instructions = new


# ---------------- helpers ----------------


def stt_int(eng, out, in0, imm, in1, op0, op1, dtype=U32):
    """scalar_tensor_tensor with integer immediate (bitvec ops)."""
    return eng.add_instruction(
        mybir.InstTensorScalarPtr(
            name=eng.bass.get_next_instruction_name(),
            is_scalar_tensor_tensor=True,
            op0=op0, op1=op1,
            ins=[eng.lower_ap(in0),
                 mybir.ImmediateValue(dtype=dtype, value=imm),
                 eng.lower_ap(in1)],
            outs=[eng.lower_ap(out)]))


def ts_int2(eng, out, in0, imm1, imm2, op0, op1, dtype=U32):
    """tensor_scalar with two integer immediates."""
    return eng.add_instruction(
        mybir.InstTensorScalarPtr(
            name=eng.bass.get_next_instruction_name(),
            op0=op0, op1=op1,
            ins=[eng.lower_ap(in0),
                 mybir.ImmediateValue(dtype=dtype, value=imm1),
                 mybir.ImmediateValue(dtype=dtype, value=imm2)],
            outs=[eng.lower_ap(out)]))


# ---------------- constants (host-side, per core half h) ----------------

_g1 = np.exp(-((np.arange(5, dtype=np.float32) - 2.0) ** 2) / 2.0)
_g1 = (_g1 / _g1.sum()).astype(np.float32)  # 5-tap gaussian
C1CONST = float(np.float32(np.tan(np.pi / 8)))  # tan(22.5 deg)
C1SQ = float(np.float32(np.float32(np.tan(np.pi / 8)) ** 2))
EPS = 1e-6

H = 1024
W = 1024
NT = 5           # tiles
SLAB = 616       # x/gray slab rows
E_N = 544        # edge-row count (17 words of 32)
NHW = 34         # halfword groups (16 e-rows each)
NHW_PAD = 40
OWN0 = 16        # owned e-row start
HYST_ITERS = 10

# packed hysteresis: [128 part = colblock(8 cols), free flat 184 u32]
# colslot 0 = left halo, 1..8 data, 9 = right halo; per colslot 18 words:
# word 0 = zero guard, 1..17 = data (bit k of word w = e-row 32*(w-1)+k)
PK_W = 18
PK_C = 10
PK_F = 184  # 10*18 = 180 data+guards, +4 zero pad for shifted views


def _op_blurv():
    M = np.zeros((H, H), np.float32)
    for r in range(H):
        for k in range(5):
            src = r + k - 2
            if src < 0:
                src = -src
            elif src >= H:
                src = 2 * (H - 1) - src
            M[r, src] += _g1[k]
    return M


def _op_sobelv(taps):
    M = np.zeros((H, H), np.float32)
    for r in range(H):
        for k in range(3):
            src = min(max(r + k - 1, 0), H - 1)
            M[r, src] += taps[k]
    return M


M_BV = _op_blurv()
M_SX = _op_sobelv([1.0, 2.0, 1.0])    # vertical part of gx
M_SY = _op_sobelv([-1.0, 0.0, 1.0])   # vertical part of gy


def make_consts(h):
    """Per-core constant arrays for half h (0=top, 1=bottom)."""
    g0 = 512 * h - 19          # global row of slab row 0

    wbv = np.zeros((NT, 128, 124), np.float32)
    wsx = np.zeros((NT, 124, 122), np.float32)
    wsy = np.zeros((NT, 124, 122), np.float32)
    for t in range(NT):
        for m in range(124):
            gout = g0 + 122 * t + 2 + m
            if not (0 <= gout < H):
                continue
            lo = max(0, gout - 2 - (g0 + 122 * t))
            for p in range(128):
                gin = g0 + 122 * t + p
                if 0 <= gin < H:
                    wbv[t, p, m] = M_BV[gout, gin]
        for m in range(122):
            gout = g0 + 122 * t + 3 + m
            if not (0 <= gout < H):
                continue
            for p in range(124):
                gin = g0 + 122 * t + 2 + p
                if 0 <= gin < H:
                    wsx[t, p, m] = M_SX[gout, gin]
                    wsy[t, p, m] = M_SY[gout, gin]

    mumask = np.ones((NT, 122, 1), np.float32)
    mdmask = np.ones((NT, 122, 1), np.float32)
    for t in range(NT):
        for p in range(122):
            g = g0 + 122 * t + 3 + p
            if g == 0:
                mumask[t, p, 0] = 0.0
            if g == H - 1:
                mdmask[t, p, 0] = 0.0

    apack = np.zeros((NT, 122, NHW), np.float32)
    for t in range(NT):
        for p in range(122):
            e = 122 * t + p
            g = g0 + 3 + e
            if 0 <= e < E_N and 0 <= g < H:
                apack[t, p, e // 16] = float(2 ** (e % 16))
    apack = apack.astype(ml_dtypes.bfloat16)

    bitsc = (2.0 ** -(np.arange(128) % 16)).astype(np.float32).reshape(128, 1)

    return dict(wbv=wbv, wsx=wsx, wsy=wsy, mumask=mumask, mdmask=mdmask,
                apack=apack, bitsc=bitsc)


def _threshold(thr):
    t32 = np.float32(thr)
    z = np.float32(t32 * t32)
    lo = np.float32(z * 0.999)
    hi = np.float32(z * 1.001)
    assert not np.sqrt(lo, dtype=np.float32) > t32
    assert np.sqrt(hi, dtype=np.float32) > t32
    while True:
        mid = np.float32((lo.astype(np.float64) + hi.astype(np.float64)) / 2)
        if mid <= lo or mid >= hi:
            break
        if np.sqrt(mid, dtype=np.float32) > t32:
            hi = mid
        else:
            lo = mid
    # hi = min z with sqrt(z) > thr; want (z > t) == (z >= hi) -> t = lo
    assert np.sqrt(hi, dtype=np.float32) > t32
    assert not np.sqrt(lo, dtype=np.float32) > t32
    assert np.nextafter(lo, hi, dtype=np.float32) == hi
    return float(lo)


T_LOW = _threshold(0.1)
T_HIGH = _threshold(0.2)

# ---------------- the kernel ----------------


def build_kernel():
    import itertools as _it

    nc = bass.Bass()

    xs = nc.dram_tensor("xs", (3, SLAB, W), F32, kind="ExternalInput")
    wbv_d = nc.dram_tensor("wbv", (NT, 128, 124), F32, kind="ExternalInput")
    wsx_d = nc.dram_tensor("wsx", (NT, 124, 122), F32, kind="ExternalInput")
    wsy_d = nc.dram_tensor("wsy", (NT, 124, 122), F32, kind="ExternalInput")
    mum_d = nc.dram_tensor("mumask", (NT, 122, 1), F32, kind="ExternalInput")
    mdm_d = nc.dram_tensor("mdmask", (NT, 122, 1), F32, kind="ExternalInput")
    ap_d = nc.dram_tensor("apack", (NT, 122, NHW), BF16, kind="ExternalInput")
    bsc_d = nc.dram_tensor("bitsc", (128, 1), F32, kind="ExternalInput")

    mag_o = nc.dram_tensor("mag", (512, W), F32, kind="ExternalOutput")
    edg_o = nc.dram_tensor("edges", (512, W), F32, kind="ExternalOutput")

    with TileContext(nc) as tc, contextlib.ExitStack() as ctx:
        _rr = _it.cycle([nc.sync, nc.scalar, nc.gpsimd])

        def dmar(dst, src):
            return next(_rr).dma_start(dst, src)

        cpool = ctx.enter_context(tc.tile_pool(name="consts", bufs=1))
        gxpool = ctx.enter_context(tc.tile_pool(name="gxy", bufs=1))
        m2pool = ctx.enter_context(tc.tile_pool(name="m2", bufs=1))
        mkpool = ctx.enter_context(tc.tile_pool(name="masks", bufs=1))
        nmpool = ctx.enter_context(tc.tile_pool(name="nms", bufs=1))
        mupool = ctx.enter_context(tc.tile_pool(name="mumd", bufs=2))
        lhpool = ctx.enter_context(tc.tile_pool(name="lh", bufs=2))
        pkpool = ctx.enter_context(tc.tile_pool(name="pk", bufs=1))

        # ---- constants to SBUF ----
        wbv = cpool.tile([128, NT * 124], F32)
        wsx = cpool.tile([124, NT * 122], F32)
        wsy = cpool.tile([124, NT * 122], F32)
        mum = cpool.tile([122, NT], F32)
        mdm = cpool.tile([122, NT], F32)
        apk = cpool.tile([122, NT * NHW], BF16)
        for t in range(NT):
            dmar(wbv[:, t * 124:(t + 1) * 124], wbv_d[t])
            dmar(wsx[:, t * 122:(t + 1) * 122], wsx_d[t])
            dmar(wsy[:, t * 122:(t + 1) * 122], wsy_d[t])
            dmar(mum[:, t:t + 1], mum_d[t])
            dmar(mdm[:, t:t + 1], mdm_d[t])
            dmar(apk[:, t * NHW:(t + 1) * NHW], ap_d[t])
        bsc = cpool.tile([128, 1], F32)
        dmar(bsc[:], bsc_d[:])
        epsb = cpool.tile([128, 1], F32)
        nc.vector.memset(epsb[:], EPS)

        m2_tiles = []
        msk_tiles = []

        # ================= phase 1: convs + masks (uses PSUM) =============
        with tc.tile_pool(name="x", bufs=2) as xpool, \
             tc.tile_pool(name="gray", bufs=1) as gpool, \
             tc.tile_pool(name="bh", bufs=1) as bpool, \
             tc.tile_pool(name="bps", bufs=2, space="PSUM") as pspool, \
             tc.tile_pool(name="gps", bufs=1, space="PSUM") as ps2pool:
            for t in range(NT):
                r0 = 122 * t
                xr = xpool.tile([128, W], F32, tag="xr")
                dmar(xr[:], xs[0, r0:r0 + 128, :])
                xg = xpool.tile([128, W], F32, tag="xg")
                dmar(xg[:], xs[1, r0:r0 + 128, :])
                xb = xpool.tile([128, W], F32, tag="xb")
                dmar(xb[:], xs[2, r0:r0 + 128, :])

                # gray [128, 1028] with reflect guard cols
                gray = gpool.tile([128, W + 4], F32, tag="gray")
                nc.scalar.mul(gray[:, 2:2 + W], xr[:], 0.299)
                nc.vector.scalar_tensor_tensor(
                    gray[:, 2:2 + W], xg[:], 0.587, gray[:, 2:2 + W],
                    Op.mult, Op.add)
                nc.vector.scalar_tensor_tensor(
                    gray[:, 2:2 + W], xb[:], 0.114, gray[:, 2:2 + W],
                    Op.mult, Op.add)
                # reflect guards: buf0 = x[2]=buf4, buf1 = x[1]=buf3,
                # buf[W+2] = x[W-2]=buf[W], buf[W+3] = x[W-3]=buf[W-1]
                nc.scalar.copy(gray[:, 0:1], gray[:, 4:5])
                nc.scalar.copy(gray[:, 1:2], gray[:, 3:4])
                nc.scalar.copy(gray[:, W + 2:W + 3], gray[:, W:W + 1])
                nc.scalar.copy(gray[:, W + 3:W + 4], gray[:, W - 1:W])

                # blur-h 5-tap -> bh [128, 1024]
                s1 = bpool.tile([128, W], F32, tag="s1")
                nc.vector.tensor_tensor(s1[:], gray[:, 0:W], gray[:, 4:4 + W],
                                        Op.add)
                s2 = bpool.tile([128, W], F32, tag="s2")
                nc.vector.tensor_tensor(s2[:], gray[:, 1:1 + W],
                                        gray[:, 3:3 + W], Op.add)
                u = bpool.tile([128, W], F32, tag="u")
                nc.vector.tensor_scalar(u[:], gray[:, 2:2 + W],
                                        float(_g1[2]), None, Op.mult)
                nc.vector.scalar_tensor_tensor(u[:], s1[:], float(_g1[0]),
                                               u[:], Op.mult, Op.add)
                bh = bpool.tile([128, W], F32, tag="bhv")
                nc.vector.scalar_tensor_tensor(bh[:], s2[:], float(_g1[1]),
                                               u[:], Op.mult, Op.add)

                # blur-v via PE -> bsb [124, 1026] with edge guard cols
                bps = pspool.tile([124, W], F32, tag="bps")
                wt = wbv[:, t * 124:(t + 1) * 124]
                nc.tensor.matmul(bps[:, 0:512], wt, bh[:, 0:512],
                                 start=True, stop=True)
                nc.tensor.matmul(bps[:, 512:1024], wt, bh[:, 512:1024],
                                 start=True, stop=True)
                bsb = gpool.tile([124, W + 2], F32, tag="bsb")
                nc.scalar.copy(bsb[:, 1:513], bps[:, 0:512])
                nc.scalar.copy(bsb[:, 513:1025], bps[:, 512:1024])
                nc.scalar.copy(bsb[:, 0:1], bsb[:, 1:2])
                nc.scalar.copy(bsb[:, W + 1:W + 2], bsb[:, W:W + 1])

                # sobel-v via PE
                gxv = ps2pool.tile([122, W], F32, tag="gxv")
                wxt = wsx[:, t * 122:(t + 1) * 122]
                nc.tensor.matmul(gxv[:, 0:512], wxt, bsb[:124, 1:513],
                                 start=True, stop=True)
                nc.tensor.matmul(gxv[:, 512:1024], wxt, bsb[:124, 513:1025],
                                 start=True, stop=True)
                gyv = ps2pool.tile([122, W], F32, tag="gyv")
                wyt = wsy[:, t * 122:(t + 1) * 122]
                nc.tensor.matmul(gyv[:, 0:512], wyt, bsb[:124, 1:513],
                                 start=True, stop=True)
                nc.tensor.matmul(gyv[:, 512:1024], wyt, bsb[:124, 513:1025],
                                 start=True, stop=True)

                # evict sobel-v psum to SBUF (walrus: one PSUM input max)
                gxs = gxpool.tile([122, W], F32, tag="gxs")
                nc.scalar.copy(gxs[:, 0:512], gxv[:, 0:512])
                nc.scalar.copy(gxs[:, 512:1024], gxv[:, 512:1024])
                gys = gxpool.tile([122, W], F32, tag="gys")
                nc.scalar.copy(gys[:, 0:512], gyv[:, 0:512])
                nc.scalar.copy(gys[:, 512:1024], gyv[:, 512:1024])
                # sobel-h
                gx = gxpool.tile([122, W], F32, tag="gx")
                nc.vector.tensor_tensor(gx[:, 1:W - 1], gxs[:, 2:W],
                                        gxs[:, 0:W - 2], Op.subtract)
                nc.vector.tensor_tensor(gx[:, 0:1], gxs[:, 1:2], gxs[:, 0:1],
                                        Op.subtract)
                nc.vector.tensor_tensor(gx[:, W - 1:W], gxs[:, W - 1:W],
                                        gxs[:, W - 2:W - 1], Op.subtract)
                u2 = nmpool.tile([122, W + 1], F32, tag="u2")
                nc.vector.tensor_tensor(u2[:, 1:W], gys[:, 0:W - 1],
                                        gys[:, 1:W], Op.add)
                nc.scalar.mul(u2[:, 0:1], gys[:, 0:1], 2.0)
                nc.scalar.mul(u2[:, W:W + 1], gys[:, W - 1:W], 2.0)
                gy = gxpool.tile([122, W], F32, tag="gy")
                nc.vector.tensor_tensor(gy[:], u2[:, 0:W], u2[:, 1:W + 1],
                                        Op.add)

                # m2 [122, 1026] with zero guard cols
                m2 = m2pool.tile([122, W + 2], F32, name=f"m2t{t}",
                                 tag=f"m2_{t}")
                nc.vector.memset(m2[:, 0:1], 0.0)
                nc.vector.memset(m2[:, W + 1:W + 2], 0.0)
                sq1 = nmpool.tile([122, W], F32, tag="sq1")
                nc.scalar.square(sq1[:], gx[:])
                sq2 = nmpool.tile([122, W], F32, tag="sq2")
                nc.scalar.square(sq2[:], gy[:])
                nc.vector.tensor_tensor(m2[:, 1:1 + W], sq1[:], sq2[:],
                                        Op.add)
                m2_tiles.append(m2)

                ish = mkpool.tile([122, W], U8, name=f"ish{t}", tag=f"ish_{t}")
                nc.vector.scalar_tensor_tensor(ish[:], sq1[:], C1SQ, sq2[:],
                                               Op.mult, Op.is_ge)
                isv = mkpool.tile([122, W], U8, name=f"isv{t}", tag=f"isv_{t}")
                nc.vector.scalar_tensor_tensor(isv[:], sq2[:], C1SQ, sq1[:],
                                               Op.mult, Op.is_ge)
                pp = nmpool.tile([122, W], F32, tag="pp")
                nc.vector.tensor_tensor(pp[:], gx[:], gy[:], Op.mult)
                sgn = mkpool.tile([122, W], U8, name=f"sgn{t}", tag=f"sgn_{t}")
                nc.vector.tensor_scalar(sgn[:], pp[:], 0.0, None, Op.is_gt)
                msk_tiles.append((ish, isv, sgn))

        # ================= phase 2: NMS per tile (+ pack accumulate) ======
        pkps_ctx = tc.tile_pool(name="packps", bufs=1, space="PSUM")
        pkps = pkps_ctx.__enter__()
        psl = pkps.tile([NHW, W], F32, tag="psl")
        psh = pkps.tile([NHW, W], F32, tag="psh")
        for t in range(NT):
            m2 = m2_tiles[t]
            ish, isv, sgn = msk_tiles[t]
            mu = mupool.tile([122, W + 2], F32, tag="mu")
            dmar(mu[1:122, :], m2[0:121, :])
            if t > 0:
                dmar(mu[0:1, :], m2_tiles[t - 1][121:122, :])
            else:
                dmar(mu[0:1, :], m2[0:1, :])  # clamp (halo edge)
            md = mupool.tile([122, W + 2], F32, tag="md")
            dmar(md[0:121, :], m2[1:122, :])
            if t < NT - 1:
                dmar(md[121:122, :], m2_tiles[t + 1][0:1, :])
            else:
                dmar(md[121:122, :], m2[121:122, :])  # clamp

            mmu = mum[:, t:t + 1]
            mmd = mdm[:, t:t + 1]
            # mask boundary rows of mu/md (image-edge zero padding)
            nc.vector.tensor_scalar(mu[:], mu[:], mmu, None, Op.mult)
            nc.vector.tensor_scalar(md[:], md[:], mmd, None, Op.mult)
            # sel starts as mxD2 = max(NW, SE) = max(mu[j-1], md[j+1])
            sel = nmpool.tile([122, W], F32, tag="sel")
            nc.vector.tensor_tensor(sel[:], mu[:, 0:W], md[:, 2:2 + W],
                                    Op.max)
            mxd1 = nmpool.tile([122, W], F32, tag="sq1")
            nc.vector.tensor_tensor(mxd1[:], mu[:, 2:2 + W], md[:, 0:W],
                                    Op.max)
            nc.vector.copy_predicated(sel[:], sgn[:], mxd1[:])
            mxv = nmpool.tile([122, W], F32, tag="sq2")
            nc.vector.tensor_tensor(mxv[:], mu[:, 1:1 + W], md[:, 1:1 + W],
                                    Op.max)
            nc.vector.copy_predicated(sel[:], isv[:], mxv[:])
            mxh = nmpool.tile([122, W], F32, tag="pp")
            nc.vector.tensor_tensor(mxh[:], m2[:, 0:W], m2[:, 2:2 + W],
                                    Op.max)
            nc.vector.copy_predicated(sel[:], ish[:], mxh[:])

            bmask = nmpool.tile([122, W], U8, tag="bmask")
            nc.vector.tensor_tensor(bmask[:], m2[:, 1:1 + W], sel[:],
                                    Op.is_gt)
            magu = nmpool.tile([122, W], F32, tag="u2")
            nc.scalar.activation(magu[:], m2[:, 1:1 + W],
                                 mybir.ActivationFunctionType.Sqrt,
                                 bias=epsb[:122, :])
            mag = nmpool.tile([122, W], F32, tag="mag")
            nc.vector.tensor_tensor(mag[:], magu[:], bmask[:], Op.mult)
            # DMA out owned rows: e=[122t,122t+122) inter [16,528)
            e0, e1 = 122 * t, 122 * t + 122
            o0, o1 = max(e0, OWN0), min(e1, OWN0 + 512)
            if o0 < o1:
                dmar(mag_o[o0 - OWN0:o1 - OWN0, :],
                                  mag[o0 - e0:o1 - e0, :])

            lt = lhpool.tile([122, W], BF16, tag="l")
            nc.vector.tensor_scalar(lt[:], mag[:], 0.1, None, Op.is_gt)
            ht = lhpool.tile([122, W], BF16, tag="h")
            nc.vector.tensor_scalar(ht[:], mag[:], 0.2, None, Op.is_gt)
            at = apk[:, t * NHW:(t + 1) * NHW]
            nc.tensor.matmul(psl[:, 0:512], at, lt[:, 0:512],
                             start=(t == 0), stop=(t == NT - 1))
            nc.tensor.matmul(psl[:, 512:1024], at, lt[:, 512:1024],
                             start=(t == 0), stop=(t == NT - 1))
            nc.tensor.matmul(psh[:, 0:512], at, ht[:, 0:512],
                             start=(t == 0), stop=(t == NT - 1))
            nc.tensor.matmul(psh[:, 512:1024], at, ht[:, 512:1024],
                             start=(t == 0), stop=(t == NT - 1))

        # ================= phase 3: convert packed counts =================
        hwl = pkpool.tile([NHW_PAD, W], U16)
        hwh = pkpool.tile([NHW_PAD, W], U16)
        nc.vector.memset(hwl[:], 0)
        nc.vector.memset(hwh[:], 0)
        nc.vector.tensor_copy(hwl[0:NHW, :], psl[:])
        nc.vector.tensor_copy(hwh[0:NHW, :], psh[:])
        pkps_ctx.__exit__(None, None, None)

        # relayout into packed [128, 184] u32
        l_pk = pkpool.tile([128, PK_F], U32)
        h_pk = pkpool.tile([128, PK_F], U32)
        nc.vector.memset(l_pk[:], 0)
        nc.vector.memset(h_pk[:], 0)
        lpk16 = l_pk[:].bitcast(U16)   # [128, 368]
        hpk16 = h_pk[:].bitcast(U16)
        with nc.allow_non_contiguous_dma("pack relayout"):
            for w in range(1, PK_W):       # data words 1..17
                for half in range(2):
                    g = 2 * (w - 1) + half
                    # dst u16 idx = 2*(18*cs + w) + half, cs = 1..8
                    dst_l = lpk16[:, 0:360].rearrange("p (c k) -> p c k", k=36)[
                        :, 1:9, 2 * w + half:2 * w + half + 1]
                    dst_h = hpk16[:, 0:360].rearrange("p (c k) -> p c k", k=36)[
                        :, 1:9, 2 * w + half:2 * w + half + 1]
                    src_l = hwl[g:g + 1, :].rearrange("one (p c) -> one p c",
                                                      c=8)
                    src_h = hwh[g:g + 1, :].rearrange("one (p c) -> one p c",
                                                      c=8)
                    dmar(dst_l, src_l)
                    dmar(dst_h, src_h)

        # w = l & ~h ; s = h (in place OK: h_pk becomes s)
        w_pk = pkpool.tile([128, PK_F], U32)
        stt_int(nc.vector, w_pk[:], h_pk[:], 0xFFFFFFFF, l_pk[:],
                Op.bitwise_xor, Op.bitwise_and)
        # zero the halo+guard slots of w (pad area may be junk from xor)
        s_pk = h_pk  # alias: strong mask, mutated in place

        # ================= phase 4: hysteresis ============================
        def pkv(x, off, c0, c1):
            """view: words [0..17) + off per colslot, colslots [c0,c1)."""
            return x[:, off:off + 180].rearrange(
                "p (c w) -> p c w", w=PK_W)[:, c0:c1, 0:17]

        hvpool = ctx.enter_context(tc.tile_pool(name="hyst", bufs=1))
        for it in range(HYST_ITERS):
            # halo refresh: left halo(slot0) <- left nbr's data col 7 (slot 8)
            dmar(s_pk[1:128, 0 * PK_W:0 * PK_W + PK_W],
                              s_pk[0:127, 8 * PK_W:8 * PK_W + PK_W])
            dmar(s_pk[0:127, 9 * PK_W:9 * PK_W + PK_W],
                              s_pk[1:128, 1 * PK_W:1 * PK_W + PK_W])
            # vertical dilate on all 10 colslots (data words 1..17)
            va = hvpool.tile([128, PK_F], U32, tag="va")
            stt_int(nc.vector, pkv(va, 1, 0, 10), pkv(s_pk, 1, 0, 10), 1,
                    pkv(s_pk, 1, 0, 10), Op.logical_shift_left, Op.bitwise_or)
            stt_int(nc.vector, pkv(va, 1, 0, 10), pkv(s_pk, 1, 0, 10), 1,
                    pkv(va, 1, 0, 10), Op.logical_shift_right, Op.bitwise_or)
            # word carries: prev word (off 0) MSB->bit0; next word (off 2)
            stt_int(nc.vector, pkv(va, 1, 0, 10), pkv(s_pk, 0, 0, 10), 31,
                    pkv(va, 1, 0, 10), Op.logical_shift_right, Op.bitwise_or)
            stt_int(nc.vector, pkv(va, 1, 0, 10), pkv(s_pk, 2, 0, 10), 31,
                    pkv(va, 1, 0, 10), Op.logical_shift_left, Op.bitwise_or)
            # horizontal dilate (data slots 1..8 read neighbors 0..9)
            hd = hvpool.tile([128, PK_F], U32, tag="hd")
            nc.vector.tensor_tensor(pkv(hd, 1, 1, 9),
                                    pkv(va, 1, 0, 8), pkv(va, 1, 1, 9),
                                    Op.bitwise_or)
            nc.vector.tensor_tensor(pkv(hd, 1, 1, 9),
                                    pkv(hd, 1, 1, 9), pkv(va, 1, 2, 10),
                                    Op.bitwise_or)
            # s |= w & hd
            tt = hvpool.tile([128, PK_F], U32, tag="tt")
            nc.vector.tensor_tensor(pkv(tt, 1, 1, 9), pkv(w_pk, 1, 1, 9),
                                    pkv(hd, 1, 1, 9), Op.bitwise_and)
            nc.vector.tensor_tensor(pkv(s_pk, 1, 1, 9), pkv(s_pk, 1, 1, 9),
                                    pkv(tt, 1, 1, 9), Op.bitwise_or)

        # ================= phase 5: unpack + edges out ====================
        hw_s = pkpool.tile([NHW_PAD, W], U16)
        nc.vector.memset(hw_s[:], 0)
        spk16 = s_pk[:].bitcast(U16)
        with nc.allow_non_contiguous_dma("unpack relayout"):
            for w in range(1, PK_W):
                for half in range(2):
                    g = 2 * (w - 1) + half
                    src = spk16[:, 0:360].rearrange("p (c k) -> p c k", k=36)[
                        :, 1:9, 2 * w + half:2 * w + half + 1]
                    dst = hw_s[g:g + 1, :].rearrange("one (p c) -> one p c",
                                                     c=8)
                    dmar(dst, src)

        s16pool = ctx.enter_context(tc.tile_pool(name="s16", bufs=2))
        for k in range(16):
            s16 = s16pool.tile([NHW_PAD, W], U16, tag="s16")
            ts_int2(nc.vector, s16[:], hw_s[:], k, 1,
                    Op.logical_shift_right, Op.bitwise_and, dtype=U16)
            s16f = s16pool.tile([NHW_PAD, W], F32, tag="s16f")
            nc.vector.tensor_copy(s16f[:], s16[:])
            # rows k (mod 16) of edges, all 4 out-tiles in one DMA:
            # src partition g = 1+8*tp+r  ->  dst row 128*tp + 16*r + k
            dst = edg_o[:].rearrange("(tp r k) w -> k tp r w", k=16, r=8)[k]
            dmar(dst, s16f[1:33, :])

    _split_multi_waits(nc)
    nc.finalize()
    return nc


def make_in_map(x_full, b, h):
    """x_full: (4,3,1024,1024) f32. Returns in_map for core (b,h)."""
    g0 = 512 * h - 19
    idx = np.clip(np.arange(g0, g0 + SLAB), 0, H - 1)
    xs = np.ascontiguousarray(x_full[b][:, idx, :], dtype=np.float32)
    c = make_consts(h)
    return {
        "xs": xs,
        "wbv": c["wbv"], "wsx": c["wsx"], "wsy": c["wsy"],
        "mumask": c["mumask"], "mdmask": c["mdmask"],
        "apack": np.asarray(c["apack"]), "bitsc": c["bitsc"],
    }



# ---------------- public entry point ----------------
from concourse.bass_utils import run_bass_kernel_spmd  # noqa: E402

_NC = None


def _get_nc():
    global _NC
    if _NC is None:
        _NC = build_kernel()
    return _NC


def kernel(x):
    """Full Canny: x (4,3,1024,1024) f32 -> (magnitude, edges) each
    (4,1,1024,1024) f32. Shards batch*halves across 8 NeuronCores."""
    x = np.asarray(x, dtype=np.float32)
    nc = _get_nc()
    in_maps = [make_in_map(x, c // 2, c % 2) for c in range(8)]
    res = run_bass_kernel_spmd(nc, in_maps, core_ids=list(range(8)))
    mag = np.zeros((4, 1, H, W), np.float32)
    edges = np.zeros((4, 1, H, W), np.float32)
    for c in range(8):
        b, h = c // 2, c % 2
        r = res.results[c]
        mag[b, 0, 512 * h:512 * h + 512] = r["mag"]
        edges[b, 0, 512 * h:512 * h + 512] = r["edges"]
    return mag, edges



# revision 26
# speedup vs baseline: 1.7998x; 1.7998x over previous
"""Canny edge detection Bass kernel for TRN2 — per-core half-image slab.

Layout: rows on partitions, cols on free axis. Per core: image b = core//2,
half h = core%2. Input slab xs [3, 616, 1024] f32 (global rows
clip(512h-19 .. 512h+597)). 5 tiles; tile t: x rows slab [122t, 122t+128),
edge-rows e = 122t + p (p in [0,122)), global g = 512h - 16 + e,
owned e in [16, 528).

Pipeline per tile:
  PE: gray+blur-v fused (6 f32 matmuls, per-channel-scaled blur-v weights)
  Act: evict -> blur-h (Pool STT chain) -> PE sobel-v (4 f32 matmuls)
  Act evicts; sobel-h (Pool TT); masks (Act squares + Pool STT); NMS
  (Pool maxes + DVE cpred chain); thresholds (STT); Act sqrt(+eps) -> mag.
Pack lt/ht bitplanes via PE matmul into [34,1024] u16 words
(bit k of word w = edge-row 16w+k); 5 bitwise hysteresis iterations
(DVE ts2-shifts / TT or-and, carry rows via SP DMAs); unpack 16 bitplanes.
"""
import contextlib

import numpy as np
import ml_dtypes

import concourse.bass as bass
import concourse.mybir as mybir
import concourse.tile as tile_mod
from concourse.tile import TileContext
from concourse.mybir import AluOpType as Op
from concourse.vector_clock import ScopedClock as _SC

F32 = mybir.dt.float32
F16 = mybir.dt.float16
BF16 = mybir.dt.bfloat16
U8 = mybir.dt.uint8
U16 = mybir.dt.uint16
U32 = mybir.dt.uint32

# ---------------- drain-split patch (walrus 1-wait limit) ----------------


def _patched_drain_and_barrier(self, tick_clock, wait_clock):
    drain_inst = self.nc.sync.drain()
    wait_clock.add_sem_waits(drain_inst.ins, _SC({None: tick_clock.global_clock}))
    si = drain_inst.ins.sync_info
    if si is not None and si.on_wait and len(si.on_wait) > 1:
        waits = list(si.on_wait)
        si.on_wait = waits[:1]
        for w in waits[1:]:
            nop = self.nc.sync.nop(nofuse=True)
            nop.ins.sync_info = mybir.SyncInfo(on_wait=[w], on_update=[])
    self.nc.all_engine_barrier()
    assert self.sems is not None
    popped = self.nc._tile_sem_poison_stack.pop()
    assert popped is self._sem_poison
    self.nc.clear_and_free_semaphores(list(self.sems.allocated().values()))
    self.nc.all_engine_barrier()


tile_mod.TileContext._drain_and_barrier = _patched_drain_and_barrier

# ---------------- multi-wait splitter (walrus allows 1 wait/inst) -------
import bass_rust as _br  # noqa: E402

_WSPLIT_CNT = [0]


def _split_multi_waits(nc):
    for f in nc.m.functions:
        for blk in f.blocks:
            new = []
            changed = False
            for ins in blk.instructions:
                si = ins.sync_info
                if si is not None and si.on_wait and len(si.on_wait) > 1:
                    waits = list(si.on_wait)
                    for w in waits[:-1]:
                        nop = _br.InstNoOp(
                            name=f"I-wsplit{_WSPLIT_CNT[0]}", ins=[], outs=[])
                        _WSPLIT_CNT[0] += 1
                        nop.engine = ins.engine
                        nop.bass_nofuse = True
                        nop.sync_info = mybir.SyncInfo(on_wait=[w],
                                                       on_update=[])
                        nc.register_instruction(nop, overwrite=True)
                        new.append(nop)
                    si.on_wait = waits[-1:]
                    changed = True
                new.append(ins)
            if changed:
                blk.instructions = new


# ---------------- helpers ----------------


def stt_int(eng, out, in0, imm, in1, op0, op1, dtype=U32):
    """scalar_tensor_tensor with integer immediate (bitvec ops)."""
    return eng.add_instruction(
        mybir.InstTensorScalarPtr(
            name=eng.bass.get_next_instruction_name(),
            is_scalar_tensor_tensor=True,
            op0=op0, op1=op1,
            ins=[eng.lower_ap(in0),
                 mybir.ImmediateValue(dtype=dtype, value=imm),
                 eng.lower_ap(in1)],
            outs=[eng.lower_ap(out)]))


def ts_int2(eng, out, in0, imm1, imm2, op0, op1, dtype=U32):
    """tensor_scalar with two integer immediates."""
    return eng.add_instruction(
        mybir.InstTensorScalarPtr(
            name=eng.bass.get_next_instruction_name(),
            op0=op0, op1=op1,
            ins=[eng.lower_ap(in0),
                 mybir.ImmediateValue(dtype=dtype, value=imm1),
                 mybir.ImmediateValue(dtype=dtype, value=imm2)],
            outs=[eng.lower_ap(out)]))


# ---------------- constants (host-side, per core half h) ----------------

_g1 = np.exp(-((np.arange(5, dtype=np.float32) - 2.0) ** 2) / 2.0)
_g1 = (_g1 / _g1.sum()).astype(np.float32)  # 5-tap gaussian
C1SQ = float(np.float32(np.float32(np.tan(np.pi / 8)) ** 2))
EPS = np.float32(1e-6)
GRAYW = (0.299, 0.587, 0.114)

H = 1024
W = 1024
NT = 5           # tiles
SLAB = 616       # x slab rows
E_N = 544        # edge-row count
NWORD = 34       # u16 words of 16 e-rows
OWN0 = 16        # owned e-row start
HYST_ITERS = 4


def _op_blurv():
    M = np.zeros((H, H), np.float32)
    for r in range(H):
        for k in range(5):
            src = r + k - 2
            if src < 0:
                src = -src
            elif src >= H:
                src = 2 * (H - 1) - src
            M[r, src] += _g1[k]
    return M


def _op_sobelv(taps):
    M = np.zeros((H, H), np.float32)
    for r in range(H):
        for k in range(3):
            src = min(max(r + k - 1, 0), H - 1)
            M[r, src] += taps[k]
    return M


M_BV = _op_blurv()
M_SX = _op_sobelv([1.0, 2.0, 1.0])    # vertical part of gx
M_SY = _op_sobelv([-1.0, 0.0, 1.0])   # vertical part of gy


def make_consts(h):
    """Per-core constant arrays for half h (0=top, 1=bottom)."""
    g0 = 512 * h - 19          # global row of slab row 0

    wbv = np.zeros((NT, 128, 124), np.float32)
    wsx = np.zeros((NT, 124, 122), np.float32)
    wsy = np.zeros((NT, 124, 122), np.float32)
    for t in range(NT):
        for m in range(124):
            gout = g0 + 122 * t + 2 + m
            if not (0 <= gout < H):
                continue
            for p in range(128):
                gin = g0 + 122 * t + p
                if 0 <= gin < H:
                    wbv[t, p, m] = M_BV[gout, gin]
        for m in range(122):
            gout = g0 + 122 * t + 3 + m
            if not (0 <= gout < H):
                continue
            for p in range(124):
                gin = g0 + 122 * t + 2 + p
                if 0 <= gin < H:
                    wsx[t, p, m] = M_SX[gout, gin]
                    wsy[t, p, m] = M_SY[gout, gin]

    # fused gray+blurv: [128, NT*3*124], col block (t*3+c)*124
    wgv = np.zeros((128, NT * 3 * 124), np.float32)
    for t in range(NT):
        for c in range(3):
            wgv[:, (t * 3 + c) * 124:(t * 3 + c + 1) * 124] = \
                wbv[t] * np.float32(GRAYW[c]) * _g1[2]
    wsx_f = np.ascontiguousarray(
        wsx.transpose(1, 0, 2).reshape(124, NT * 122))
    wsy_f = np.ascontiguousarray(
        wsy.transpose(1, 0, 2).reshape(124, NT * 122))

    # pack weights: apk[p, t*34 + w] = 2^k where 122t+p = 16w + k
    apk = np.zeros((122, NT * NWORD), np.float32)
    for t in range(NT):
        for p in range(122):
            e = 122 * t + p
            if e < E_N:
                apk[p, t * NWORD + e // 16] = float(2 ** (e % 16))
    apk = apk.astype(np.float16)

    return dict(wgv=wgv, wsx=wsx_f, wsy=wsy_f, apk=apk)


def _threshold_m2(thr):
    """Largest f32 m with sqrt(f32(m + EPS)) <= thr; compare m2 > T."""
    t32 = np.float32(thr)
    lo = np.float32(t32 * t32 * 0.99)
    hi = np.float32(t32 * t32 * 1.01)
    assert not np.sqrt(np.float32(lo + EPS), dtype=np.float32) > t32
    assert np.sqrt(np.float32(hi + EPS), dtype=np.float32) > t32
    while True:
        mid = np.float32((lo.astype(np.float64) + hi.astype(np.float64)) / 2)
        if mid <= lo or mid >= hi:
            break
        if np.sqrt(np.float32(mid + EPS), dtype=np.float32) > t32:
            hi = mid
        else:
            lo = mid
    assert np.nextafter(lo, hi, dtype=np.float32) == hi
    return float(lo)


T2L = _threshold_m2(0.1)
T2H = _threshold_m2(0.2)

# ---------------- the kernel ----------------


def build_kernel():
    nc = bass.Bass()

    xs = nc.dram_tensor("xs", (3, SLAB, W), F32, kind="ExternalInput")
    wgv_d = nc.dram_tensor("wgv", (128, NT * 3 * 124), F32,
                           kind="ExternalInput")
    wsx_d = nc.dram_tensor("wsx", (124, NT * 122), F32, kind="ExternalInput")
    wsy_d = nc.dram_tensor("wsy", (124, NT * 122), F32, kind="ExternalInput")
    apk_d = nc.dram_tensor("apk", (122, NT * NWORD), F16,
                           kind="ExternalInput")

    mag_o = nc.dram_tensor("mag", (512, W), F16, kind="ExternalOutput")
    m2buf = nc.dram_tensor("m2buf", (NT * 122 + 2, W + 2), F32,
                           kind="Internal")
    edg_o = nc.dram_tensor("edg16", (4, 32, W), U16, kind="ExternalOutput")

    with TileContext(nc) as tc, contextlib.ExitStack() as ctx:
        cpool = ctx.enter_context(tc.tile_pool(name="consts", bufs=1))

        # ---- constants to SBUF ----
        wgv = cpool.tile([128, NT * 3 * 124], F32)
        nc.gpsimd.dma_start(wgv[:], wgv_d[:])
        wsx = cpool.tile([124, NT * 122], F32)
        nc.gpsimd.dma_start(wsx[:], wsx_d[:])
        wsy = cpool.tile([124, NT * 122], F32)
        nc.gpsimd.dma_start(wsy[:], wsy_d[:])
        apk = cpool.tile([122, NT * NWORD], F16)
        nc.gpsimd.dma_start(apk[:], apk_d[:])
        epsb = cpool.tile([128, 1], F32)
        nc.gpsimd.memset(epsb[:], float(EPS))
        zrow = cpool.tile([1, W + 2], F32)
        nc.gpsimd.memset(zrow[:], 0.0)
        nc.scalar.dma_start(m2buf[0:1, :], zrow[:])
        nc.scalar.dma_start(m2buf[NT * 122 + 1:NT * 122 + 2, :], zrow[:])

        pkps_pool = ctx.enter_context(tc.tile_pool(name="pkps", bufs=1,
                                                    space="PSUM"))
        psl = pkps_pool.tile([98, 512], F32)
        psh = pkps_pool.tile([98, 512], F32)
        nc.vector.memset(psl[:], 0.0)
        nc.vector.memset(psh[:], 0.0)

        mainctx = contextlib.ExitStack()
        xpool = mainctx.enter_context(tc.tile_pool(name="x", bufs=2))
        bpool = mainctx.enter_context(tc.tile_pool(name="b", bufs=2))
        gpool = mainctx.enter_context(tc.tile_pool(name="g", bufs=2))
        g1pool = mainctx.enter_context(tc.tile_pool(name="g1", bufs=1))
        m2pool = mainctx.enter_context(tc.tile_pool(name="m2", bufs=1))
        mkpool = mainctx.enter_context(tc.tile_pool(name="mk", bufs=1))
        nmpool = mainctx.enter_context(tc.tile_pool(name="nm", bufs=1))
        ltpool = mainctx.enter_context(tc.tile_pool(name="lt", bufs=2))
        pspool = mainctx.enter_context(tc.tile_pool(name="ps", bufs=2,
                                                     space="PSUM"))
        psbpool = mainctx.enter_context(tc.tile_pool(name="psb", bufs=1,
                                                     space="PSUM"))
        m2_tiles = [None] * NT
        msk_tiles = [None] * NT

        def conv_tile(t):
            r0 = 122 * t
            # ---- input DMAs ----
            xr = xpool.tile([128, W], F32, tag="xr")
            nc.sync.dma_start(xr[:], xs[0, r0:r0 + 128, :])
            xg = xpool.tile([128, W], F32, tag="xg")
            nc.sync.dma_start(xg[:], xs[1, r0:r0 + 128, :])
            xb = xpool.tile([128, W], F32, tag="xb")
            nc.scalar.dma_start(xb[:], xs[2, r0:r0 + 128, :])

            # ---- gray + blur-v via PE (f32) ----
            bvps = pspool.tile([124, W], F32, tag="psA")
            for ci, xc in enumerate((xr, xg, xb)):
                wt = wgv[:, (t * 3 + ci) * 124:(t * 3 + ci + 1) * 124]
                nc.tensor.matmul(bvps[:, 0:512], wt, xc[:, 0:512],
                                 start=(ci == 0), stop=(ci == 2))
                nc.tensor.matmul(bvps[:, 512:1024], wt, xc[:, 512:1024],
                                 start=(ci == 0), stop=(ci == 2))

            # evict with reflect guard cols: bvs[j] = bv[j-2]
            bvs = bpool.tile([124, W + 4], F32, tag="bvs")
            nc.scalar.copy(bvs[:, 2:2 + W], bvps[:])
            nc.gpsimd.tensor_copy(bvs[:, 0:1], bvs[:, 4:5])
            nc.gpsimd.tensor_copy(bvs[:, 1:2], bvs[:, 3:4])
            nc.gpsimd.tensor_copy(bvs[:, W + 2:W + 3], bvs[:, W:W + 1])
            nc.gpsimd.tensor_copy(bvs[:, W + 3:W + 4], bvs[:, W - 1:W])

            # ---- blur-h (Pool STT chain) ----
            s1 = bpool.tile([124, W], F32, tag="s1")
            nc.gpsimd.tensor_tensor(s1[:], bvs[:, 0:W], bvs[:, 4:4 + W],
                                    Op.add)
            s2 = bpool.tile([124, W], F32, tag="s2")
            nc.gpsimd.tensor_tensor(s2[:], bvs[:, 1:1 + W], bvs[:, 3:3 + W],
                                    Op.add)
            t2 = bpool.tile([124, W], F32, tag="t2")
            nc.vector.scalar_tensor_tensor(t2[:], s2[:],
                                           float(np.float32(_g1[1] / _g1[2])),
                                           bvs[:, 2:2 + W], Op.mult, Op.add)
            bh = bpool.tile([124, W], F32, tag="bh")
            nc.vector.scalar_tensor_tensor(bh[:], s1[:],
                                           float(np.float32(_g1[0] / _g1[2])),
                                           t2[:], Op.mult, Op.add)

            # ---- sobel-v via PE (f32) ----
            gxps = bvps
            wxt = wsx[:, t * 122:(t + 1) * 122]
            nc.tensor.matmul(gxps[0:122, 0:512], wxt, bh[:, 0:512],
                             start=True, stop=True)
            nc.tensor.matmul(gxps[0:122, 512:1024], wxt, bh[:, 512:1024],
                             start=True, stop=True)
            gyps = psbpool.tile([122, W], F32, tag="psB")
            wyt = wsy[:, t * 122:(t + 1) * 122]
            nc.tensor.matmul(gyps[:, 0:512], wyt, bh[:, 0:512],
                             start=True, stop=True)
            nc.tensor.matmul(gyps[:, 512:1024], wyt, bh[:, 512:1024],
                             start=True, stop=True)

            # evict with replicate guard cols: gxs[j] = gxv[j-1]
            gxs = gpool.tile([122, W + 2], F32, tag="gxs")
            nc.scalar.copy(gxs[:, 1:1 + W], gxps[0:122, :])
            nc.gpsimd.tensor_copy(gxs[:, 0:1], gxs[:, 1:2])
            nc.gpsimd.tensor_copy(gxs[:, W + 1:W + 2], gxs[:, W:W + 1])
            gys = gpool.tile([122, W + 2], F32, tag="gys")
            nc.scalar.copy(gys[:, 1:1 + W], gyps[:, :])
            nc.gpsimd.tensor_copy(gys[:, 0:1], gys[:, 1:2])
            nc.gpsimd.tensor_copy(gys[:, W + 1:W + 2], gys[:, W:W + 1])

            # ---- sobel-h ----
            gx = g1pool.tile([122, W], F32, tag="gx")
            nc.vector.tensor_tensor(gx[:], gxs[:, 2:2 + W], gxs[:, 0:W],
                                    Op.subtract)
            u2 = g1pool.tile([122, W + 1], F32, tag="u2")
            nc.vector.tensor_tensor(u2[:], gys[:, 0:W + 1], gys[:, 1:W + 2],
                                    Op.add)
            gy = g1pool.tile([122, W], F32, tag="gy")
            nc.gpsimd.tensor_tensor(gy[:], u2[:, 0:W], u2[:, 1:W + 1],
                                    Op.add)

            # ---- masks (squares on Act; STT compares on Pool) ----
            sq1 = mkpool.tile([122, W], F32, tag="sq1")
            nc.gpsimd.tensor_tensor(sq1[:], gx[:], gx[:], Op.mult)
            sq2 = mkpool.tile([122, W], F32, tag="sq2")
            nc.gpsimd.tensor_tensor(sq2[:], gy[:], gy[:], Op.mult)
            ish = mkpool.tile([122, W], U16, tag=f"ish{t % 2}")
            nc.vector.scalar_tensor_tensor(ish[:], sq1[:], C1SQ, sq2[:],
                                           Op.mult, Op.is_ge)
            isv = mkpool.tile([122, W], U16, tag=f"isv{t % 2}")
            nc.vector.scalar_tensor_tensor(isv[:], sq2[:], C1SQ, sq1[:],
                                           Op.mult, Op.is_ge)
            pp = mkpool.tile([122, W], F16, tag="pp")
            nc.gpsimd.tensor_tensor(pp[:], gx[:], gy[:], Op.mult)
            sgn = mkpool.tile([122, W], U16, tag=f"sgn{t % 2}")
            nc.vector.tensor_scalar(sgn[:], pp[:], 0.0, None, Op.is_gt)

            # ---- m2 with zero guard cols ----
            m2 = m2pool.tile([122, W + 2], F32, name=f"m2t{t}",
                             tag=f"m2_{t % 3}")
            nc.gpsimd.memset(m2[:, 0:1], 0.0)
            nc.gpsimd.memset(m2[:, W + 1:W + 2], 0.0)
            nc.vector.tensor_tensor(m2[:, 1:1 + W], sq1[:], sq2[:], Op.add)
            nc.sync.dma_start(m2buf[1 + 122 * t:1 + 122 * t + 122, :],
                              m2[:, :])
            m2_tiles[t] = m2
            msk_tiles[t] = (ish, isv, sgn)

        def nms_tile(t):
            m2 = m2_tiles[t]
            ish, isv, sgn = msk_tiles[t]
            # ---- mu/md row-shifted windows from HBM m2 staging ----
            mu = nmpool.tile([122, W + 2], F32, tag="mu")
            nc.sync.dma_start(mu[:, :], m2buf[122 * t:122 * t + 122, :])
            md = nmpool.tile([122, W + 2], F32, tag="md")
            nc.scalar.dma_start(md[:, :], m2buf[2 + 122 * t:2 + 122 * t + 122, :])

            # ---- NMS: 4 direction maxes + cpred select chain ----
            sel = nmpool.tile([122, W], F32, tag="sel")
            nc.vector.tensor_tensor(sel[:], mu[:, 0:W], md[:, 2:2 + W],
                                    Op.max)     # NW/SE pair (mxd2)
            d1 = nmpool.tile([122, W], F32, tag="d1")
            nc.gpsimd.tensor_tensor(d1[:], mu[:, 2:2 + W], md[:, 0:W],
                                    Op.subtract)
            r1 = nmpool.tile([122, W], F32, tag="r1")
            nc.scalar.activation(r1[:], d1[:],
                                 mybir.ActivationFunctionType.Relu)
            mxd1 = nmpool.tile([122, W], F32, tag="mxd1")
            nc.gpsimd.tensor_tensor(mxd1[:], md[:, 0:W], r1[:],
                                    Op.add)     # NE/SW pair via relu-max
            d2 = nmpool.tile([122, W], F32, tag="d2")
            nc.gpsimd.tensor_tensor(d2[:], mu[:, 1:1 + W], md[:, 1:1 + W],
                                    Op.subtract)
            r2 = nmpool.tile([122, W], F32, tag="r2")
            nc.scalar.activation(r2[:], d2[:],
                                 mybir.ActivationFunctionType.Relu)
            mxv = nmpool.tile([122, W], F32, tag="mxv")
            nc.gpsimd.tensor_tensor(mxv[:], md[:, 1:1 + W], r2[:], Op.add)
            mxh = nmpool.tile([122, W], F32, tag="mxh")
            nc.vector.tensor_tensor(mxh[:], m2[:, 0:W], m2[:, 2:2 + W],
                                    Op.max)
            nc.vector.copy_predicated(sel[:], sgn[:], mxd1[:])
            nc.vector.copy_predicated(sel[:], isv[:], mxv[:])
            nc.vector.copy_predicated(sel[:], ish[:], mxh[:])

            # ---- is_max, thresholds, mag ----
            bmask = nmpool.tile([122, W], F16, tag="bmask")
            nc.vector.tensor_tensor(bmask[:], m2[:, 1:1 + W], sel[:],
                                    Op.is_gt)
            m2m = nmpool.tile([122, W], F32, tag="m2m")
            nc.gpsimd.tensor_tensor(m2m[:], m2[:, 1:1 + W], bmask[:],
                                    Op.mult)
            lt = ltpool.tile([122, W], F16, tag="lt")
            nc.vector.tensor_scalar(lt[:], m2m[:], T2L, None, Op.is_gt)
            ht = ltpool.tile([122, W], F16, tag="ht")
            nc.vector.tensor_scalar(ht[:], m2m[:], T2H, None, Op.is_gt)
            mag0 = nmpool.tile([122, W], F16, tag="mag0")
            nc.scalar.activation(mag0[:], m2[:, 1:1 + W],
                                 mybir.ActivationFunctionType.Sqrt,
                                 bias=epsb[:122, :])
            mag = nmpool.tile([122, W], F16, tag="mag")
            nc.vector.tensor_tensor(mag[:], mag0[:], bmask[:], Op.mult)

            # DMA out owned rows: e=[122t,122t+122) inter [16,528)
            e0, e1 = 122 * t, 122 * t + 122
            o0, o1 = max(e0, OWN0), min(e1, OWN0 + 512)
            if o0 < o1:
                nc.sync.dma_start(mag_o[o0 - OWN0:o1 - OWN0, :],
                                  mag[o0 - e0:o1 - e0, :])

            # ---- pack lt/ht into bitplane words via PE ----
            at = apk[:, t * NWORD:(t + 1) * NWORD]
            nc.tensor.matmul(psl[0:NWORD, :], at, lt[:, 0:512],
                             start=False, stop=False,
                             skip_group_check=True)
            nc.tensor.matmul(psl[64:64 + NWORD, :], at, lt[:, 512:1024],
                             start=False, stop=(t == NT - 1),
                             skip_group_check=True)
            nc.tensor.matmul(psh[0:NWORD, :], at, ht[:, 0:512],
                             start=False, stop=False,
                             skip_group_check=True)
            nc.tensor.matmul(psh[64:64 + NWORD, :], at, ht[:, 512:1024],
                             start=False, stop=(t == NT - 1),
                             skip_group_check=True)

        for t in range(NT):
            conv_tile(t)
            if t >= 1:
                nms_tile(t - 1)
        nms_tile(NT - 1)
        mainctx.close()

        # ================= hysteresis (bit-packed u16) ====================
        # Layout [98, 524]: partitions 0:34 = words of cols 0:512 ("block A"),
        # partitions 64:98 = words of cols 512:1024 ("block B"); col j holds
        # global col (j-6) for A / (506+j) for B -> 6-col halos, no per-iter
        # seam exchange (exact for <=5 iterations).
        BH = 6
        WB = 512 + 2 * BH
        hpool = ctx.enter_context(tc.tile_pool(name="hy", bufs=1))
        l_pk = hpool.tile([98, WB], U16)
        nc.vector.memset(l_pk[:], 0)
        s_a = hpool.tile([98, WB], U16)
        nc.vector.memset(s_a[:], 0)
        nc.vector.tensor_copy(l_pk[0:NWORD, BH:BH + 512], psl[0:NWORD, :])
        nc.scalar.copy(l_pk[64:64 + NWORD, BH:BH + 512], psl[64:64 + NWORD, :])
        nc.vector.tensor_copy(s_a[0:NWORD, BH:BH + 512], psh[0:NWORD, :])
        nc.scalar.copy(s_a[64:64 + NWORD, BH:BH + 512], psh[64:64 + NWORD, :])
        # seam halos (one-time; halo then self-updates within each block)
        for tt in (l_pk, s_a):
            nc.sync.dma_start(tt[0:NWORD, 512 + BH:512 + 2 * BH],
                              tt[64:64 + NWORD, BH:2 * BH])
            nc.sync.dma_start(tt[64:64 + NWORD, 0:BH],
                              tt[0:NWORD, 512:512 + BH])
        s_b = hpool.tile([98, WB], U16)
        vd = hpool.tile([98, WB + 2], U16)
        nc.gpsimd.memset(vd[:, 0:1], 0)
        nc.gpsimd.memset(vd[:, WB + 1:WB + 2], 0)
        cbu = hpool.tile([98, WB], U16)
        nc.gpsimd.memset(cbu[:], 0)
        cbd = hpool.tile([98, WB], U16)
        nc.gpsimd.memset(cbd[:], 0)
        tpool = ctx.enter_context(tc.tile_pool(name="ht2", bufs=2))

        s_cur, s_nxt = s_a, s_b
        for it in range(HYST_ITERS):
            # word carries: cbu[w] = s[w-1], cbd[w] = s[w+1] (per block)
            nc.sync.dma_start(cbu[1:NWORD, :], s_cur[0:NWORD - 1, :])
            nc.sync.dma_start(cbu[65:64 + NWORD, :],
                              s_cur[64:64 + NWORD - 1, :])
            nc.sync.dma_start(cbd[0:NWORD - 1, :], s_cur[1:NWORD, :])
            nc.sync.dma_start(cbd[64:64 + NWORD - 1, :],
                              s_cur[65:64 + NWORD, :])
            sl = tpool.tile([98, WB], U16, tag="sl")
            ts_int2(nc.vector, sl[:], s_cur[:], 1, 0xFFFF,
                    Op.logical_shift_left, Op.bitwise_and, dtype=U16)
            sr = tpool.tile([98, WB], U16, tag="sr")
            ts_int2(nc.vector, sr[:], s_cur[:], 1, 0xFFFF,
                    Op.logical_shift_right, Op.bitwise_and, dtype=U16)
            cu = tpool.tile([98, WB], U16, tag="cu")
            ts_int2(nc.vector, cu[:], cbu[:], 15, 0xFFFF,
                    Op.logical_shift_right, Op.bitwise_and, dtype=U16)
            cd = tpool.tile([98, WB], U16, tag="cd")
            ts_int2(nc.vector, cd[:], cbd[:], 15, 0xFFFF,
                    Op.logical_shift_left, Op.bitwise_and, dtype=U16)
            va = tpool.tile([98, WB], U16, tag="va")
            nc.vector.tensor_tensor(va[:], s_cur[:], sl[:], Op.bitwise_or)
            vb = tpool.tile([98, WB], U16, tag="vb")
            nc.vector.tensor_tensor(vb[:], sr[:], cu[:], Op.bitwise_or)
            vc = tpool.tile([98, WB], U16, tag="vc")
            nc.vector.tensor_tensor(vc[:], va[:], vb[:], Op.bitwise_or)
            nc.vector.tensor_tensor(vd[:, 1:1 + WB], vc[:], cd[:],
                                    Op.bitwise_or)
            hd = tpool.tile([98, WB], U16, tag="hd")
            nc.vector.tensor_tensor(hd[:], vd[:, 0:WB], vd[:, 2:2 + WB],
                                    Op.bitwise_or)
            hd2 = tpool.tile([98, WB], U16, tag="hd2")
            nc.vector.tensor_tensor(hd2[:], hd[:], vd[:, 1:1 + WB],
                                    Op.bitwise_or)
            nc.vector.tensor_tensor(s_nxt[:], l_pk[:], hd2[:],
                                    Op.bitwise_and)
            s_cur, s_nxt = s_nxt, s_cur

        # ================= unpack bitplanes + edges out ===================
        upool = ctx.enter_context(tc.tile_pool(name="up", bufs=4))
        for g in range(4):
            s16 = upool.tile([98, WB], U16, tag="s16")
            ts_int2(nc.vector, s16[:], s_cur[:], 4 * g, 0xF,
                    Op.logical_shift_right, Op.bitwise_and, dtype=U16)
            eng = (nc.sync, nc.scalar)[g % 2]
            eng.dma_start(edg_o[g][:, 0:512], s16[1:33, BH:BH + 512])
            eng2 = (nc.scalar, nc.sync)[g % 2]
            eng2.dma_start(edg_o[g][:, 512:1024],
                           s16[65:97, BH:BH + 512])

    _split_multi_waits(nc)
    nc.finalize()
    return nc


def make_in_map(x_full, b, h):
    """x_full: (4,3,1024,1024) f32. Returns in_map for core (b,h)."""
    g0 = 512 * h - 19
    idx = np.clip(np.arange(g0, g0 + SLAB), 0, H - 1)
    xs = np.ascontiguousarray(x_full[b][:, idx, :], dtype=np.float32)
    c = make_consts(h)
    return {
        "xs": xs,
        "wgv": c["wgv"], "wsx": c["wsx"], "wsy": c["wsy"],
        "apk": c["apk"],
    }


# ---------------- public entry point ----------------
from concourse.bass_utils import run_bass_kernel_spmd  # noqa: E402

_NC = None


def _get_nc():
    global _NC
    if _NC is None:
        _NC = build_kernel()
    return _NC


def kernel(x):
    """Full Canny: x (4,3,1024,1024) f32 -> (magnitude, edges) each
    (4,1,1024,1024) f32. Shards batch*halves across 8 NeuronCores."""
    x = np.asarray(x, dtype=np.float32)
    nc = _get_nc()
    in_maps = [make_in_map(x, c // 2, c % 2) for c in range(8)]
    res = run_bass_kernel_spmd(nc, in_maps, core_ids=list(range(8)))
    mag = np.zeros((4, 1, H, W), np.float32)
    edges = np.zeros((4, 1, H, W), np.float32)
    for c in range(8):
        b, h = c // 2, c % 2
        r = res.results[c]
        mag[b, 0, 512 * h:512 * h + 512] = np.asarray(r["mag"],
                                                      np.float32)
        e4 = np.asarray(r["edg16"])           # [4, 32, 1024] u16 nibbles
        e16 = np.stack([(e4[k // 4] >> (k % 4)) & 1 for k in range(16)])
        eh = e16.transpose(1, 0, 2).reshape(512, W)
        edges[b, 0, 512 * h:512 * h + 512] = eh.astype(np.float32)
    return mag, edges
